# revision 6
# baseline (speedup 1.0000x reference)
"""Causal self-attention (B=4, T=2048, D=1024, H=16, hd=64) on 8 trn2 NeuronCores.

Sharding: data parallel over batch (4) x tensor parallel over heads (2 groups
of 8). Core c handles batch c//2 and heads (c%2)*8 .. (c%2)*8+8.
Wq/Wk/Wv are column-parallel by head group, Wo row-parallel; the pair of
cores sharing a batch produce partial outputs that are summed on the host.

On-device layout (per core) is fully "transposed": projections produce
Q^T, K^T [512, 2048] and V [2048, 512], scores are computed as
S^T = K Q^T (j=key on partitions, i=query on free dim), softmax uses
exp without max subtraction (scores are O(6) here), the denominator
comes for free from a ones-column appended to V, and attention output
O^T [hd, T] feeds the row-parallel out-projection directly as lhsT.

v2 schedule: the scalar engine's exp stream (160 x ~1.1us) is the pacer.
Everything else (Q/K/V projections, out-projection) is emitted as fillers
inside the attention j-tile loop so exp starts ~10us into the run and the
PE never idles waiting for it. Exp and the S matmuls are trimmed to the
causally-valid query range of each diagonal tile.
"""

import contextlib
import ctypes
import sys
import types

import numpy as np

B, T, D = 4, 2048, 1024
H_TOT, HD = 16, 64
SCALE = HD ** -0.5
P = 128
NH = 8            # heads per core
QD = NH * HD      # 512, projected dim per core
KT = D // P       # 8 contraction tiles for projections
MT = QD // P      # 4 qdim tiles
TT = T // P       # 16 token tiles
ACH = 512         # token chunk; PSUM bank caps matmul N at 512
NACH = T // ACH   # 4
ICH = 512         # attention query chunk
NIC = T // ICH    # 4

_PROGRAM = None  # compiled program cache — build once per process


def _install_ntff_hook():
    """antenv.axon_hooks is missing in this image; recreate it so
    run_bass_kernel_spmd(trace=True) can profile. Harmless if unused."""
    if "antenv.axon_hooks" in sys.modules:
        return
    try:
        import antenv
    except ImportError:
        return
    mod = types.ModuleType("antenv.axon_hooks")
    _hook = [None]
    mod.set_axon_ntff_profile_hook = lambda h: _hook.__setitem__(0, h)
    mod.get_axon_ntff_profile_hook = lambda: _hook[0]
    antenv.axon_hooks = mod
    sys.modules["antenv.axon_hooks"] = mod
    try:
        lib = ctypes.CDLL("/opt/axon/libaxon_pjrt.so")
        if not hasattr(lib, "axon_start_nrt_profile"):
            return
        lib.axon_start_nrt_profile.argtypes = [
            ctypes.POINTER(ctypes.c_int64), ctypes.c_size_t]
        lib.axon_start_nrt_profile.restype = ctypes.c_int64
        lib.axon_stop_nrt_profile.argtypes = [ctypes.c_char_p]
        lib.axon_stop_nrt_profile.restype = ctypes.c_int64

        @contextlib.contextmanager
        def _hookfn(output_dir, device_ids):
            import jax
            jax.devices()
            if device_ids:
                ids = (ctypes.c_int64 * len(device_ids))(*device_ids)
                rc = lib.axon_start_nrt_profile(ids, len(device_ids))
            else:
                rc = lib.axon_start_nrt_profile(None, 0)
            if rc != 0:
                raise RuntimeError(f"axon_start_nrt_profile rc={rc}")
            try:
                yield
            finally:
                n = lib.axon_stop_nrt_profile(str(output_dir).encode())
                print(f"profile: {n} file(s) written to {output_dir}")

        mod.set_axon_ntff_profile_hook(_hookfn)
    except OSError:
        pass


def _build_program():
    from contextlib import ExitStack

    import concourse.tile as tile
    from concourse import bacc, mybir

    F32 = mybir.dt.float32
    BF16 = mybir.dt.bfloat16
    AF = mybir.ActivationFunctionType
    ALU = mybir.AluOpType

    nc = bacc.Bacc("TRN2", target_bir_lowering=False, debug=False,
                   num_devices=8)

    # all tensor inputs arrive pre-arranged in SBUF layout [128, k, n]
    # (host does the transpose) so every DMA is long contiguous runs
    xT_d = nc.dram_tensor("xT", [P, KT * T], BF16, kind="ExternalInput").ap()
    wq_d = nc.dram_tensor("wq", [P, KT * QD], BF16, kind="ExternalInput").ap()
    wk_d = nc.dram_tensor("wk", [P, KT * QD], BF16, kind="ExternalInput").ap()
    wv_d = nc.dram_tensor("wv", [P, KT * QD], BF16, kind="ExternalInput").ap()
    wo_d = nc.dram_tensor("wo", [P, MT * D], BF16, kind="ExternalInput").ap()
    bq_d = nc.dram_tensor("bq", [P, MT], F32, kind="ExternalInput").ap()
    bk_d = nc.dram_tensor("bk", [P, MT], F32, kind="ExternalInput").ap()
    bvb_d = nc.dram_tensor("bvb", [P, QD], F32, kind="ExternalInput").ap()
    msk_d = nc.dram_tensor("msk", [P, P], BF16, kind="ExternalInput").ap()
    out_d = nc.dram_tensor("out", [T, D], F32, kind="ExternalOutput").ap()

    xT_k = xT_d.rearrange("p (k t) -> p k t", k=KT)      # [128, 8, 2048]
    wq_k = wq_d.rearrange("p (k m) -> p k m", k=KT)      # [128, 8, 512]
    wk_k = wk_d.rearrange("p (k m) -> p k m", k=KT)
    wv_k = wv_d.rearrange("p (k m) -> p k m", k=KT)
    wo_k = wo_d.rearrange("p (k e) -> p k e", k=MT)      # [128, 4, 1024]

    with tile.TileContext(nc) as tc, ExitStack() as ctx:
        persist = ctx.enter_context(tc.tile_pool(name="persist", bufs=1))

        qt = [persist.tile([P, T], BF16, name=f"qt{i}") for i in range(MT)]
        kt_ = [persist.tile([P, T], BF16, name=f"kt{i}") for i in range(MT)]
        v3 = [persist.tile([P, NH, HD + 1], BF16, name=f"v3_{i}")
              for i in range(TT)]
        at = [persist.tile([P, T], BF16, name=f"at{i}") for i in range(MT)]
        xt_all = persist.tile([P, KT, T], BF16, name="xt")

        wq_sb = persist.tile([P, KT, QD], BF16, name="wq")
        wk_sb = persist.tile([P, KT, QD], BF16, name="wk")
        bq_sb = persist.tile([P, MT], F32, name="bq")
        bk_sb = persist.tile([P, MT], F32, name="bk")
        bvb_sb = persist.tile([P, NH, HD], F32, name="bvb")
        tri_sb = persist.tile([P, P], BF16, name="tri")
        wv_sb = persist.tile([P, KT, QD], BF16, name="wv")
        wo_sb = persist.tile([P, MT, D], BF16, name="wo")

        # DMA order tuned so the first Q/K projection (chunk 0, head-pair 0)
        # and the first V tiles can start ~7us in. xT arrives in 4 chunks
        # of 512 tokens (strided 2D transfers, 1KB runs per partition).
        nc.sync.dma_start(wq_sb[:], wq_k)
        nc.sync.dma_start(bq_sb[:], bq_d)
        nc.sync.dma_start(xt_all[:, :, 0:ACH], xT_k[:, :, 0:ACH])
        nc.sync.dma_start(wk_sb[:], wk_d.rearrange("p (k m) -> p k m", k=KT))
        nc.sync.dma_start(bk_sb[:], bk_d)
        nc.sync.dma_start(tri_sb[:], msk_d)
        nc.sync.dma_start(wv_sb[:], wv_k)
        nc.sync.dma_start(
            bvb_sb[:], bvb_d.rearrange("p (h d) -> p h d", d=HD))
        nc.sync.dma_start(xt_all[:, :, ACH:2 * ACH], xT_k[:, :, ACH:2 * ACH])
        nc.sync.dma_start(xt_all[:, :, 2 * ACH:3 * ACH],
                          xT_k[:, :, 2 * ACH:3 * ACH])
        nc.sync.dma_start(xt_all[:, :, 3 * ACH:4 * ACH],
                          xT_k[:, :, 3 * ACH:4 * ACH])
        nc.sync.dma_start(wo_sb[:], wo_k)
        for tt in range(TT):
            nc.vector.memset(v3[tt][:, :, HD:HD + 1], 1.0)

        # ---- runway: Q/K for (chunk 0, head-pair 0) in a short-lived pool
        # whose banks free up before the attention pools open ------------
        with tc.tile_pool(name="rway", bufs=1, space="PSUM") as rp:
            for w_sb, dst, b_sb in ((wq_sb, qt, bq_sb), (wk_sb, kt_, bk_sb)):
                ps = rp.tile([P, ACH], F32, name="rw", bufs=2)
                for k in range(KT):
                    nc.tensor.matmul(ps[:], w_sb[:, k, 0:P],
                                     xt_all[:, k, 0:ACH],
                                     start=(k == 0), stop=(k == KT - 1))
                nc.vector.tensor_scalar_add(dst[0][:, 0:ACH], ps[:],
                                            b_sb[:, 0:1])

        # ---- attention + fillers, one fused software-pipelined stream ----
        with tc.tile_pool(name="attnsb", bufs=1) as ap_, \
             tc.tile_pool(name="obp", bufs=3) as obp, \
             tc.tile_pool(name="attnps", bufs=1, space="PSUM") as sp:

            def emit_projqk(c, hp, which):
                """Project Q (which=0) or K (which=1) for (chunk c, hp)."""
                w_sb, dst, b_sb = ((wq_sb, qt, bq_sb) if which == 0
                                   else (wk_sb, kt_, bk_sb))
                ps = sp.tile([P, ACH], F32, name="misc", bufs=1)
                csl = slice(c * ACH, (c + 1) * ACH)
                for k in range(KT):
                    nc.tensor.matmul(ps[:], w_sb[:, k, hp * P:(hp + 1) * P],
                                     xt_all[:, k, csl],
                                     start=(k == 0), stop=(k == KT - 1))
                nc.vector.tensor_scalar_add(dst[hp][:, csl], ps[:],
                                            b_sb[:, hp:hp + 1])

            def emit_v_tile(tt):
                psv = sp.tile([P, QD], F32, name="misc", bufs=1)
                for k in range(KT):
                    nc.tensor.matmul(
                        psv[:], xt_all[:, k, tt * P:(tt + 1) * P],
                        wv_sb[:, k, :], start=(k == 0), stop=(k == KT - 1))
                nc.vector.tensor_tensor(
                    v3[tt][:, :, 0:HD],
                    psv[:].rearrange("p (h d) -> p h d", d=HD),
                    bvb_sb[:], op=ALU.add)

            def emit_out_group(mt, nch2, alt=False):
                if alt:
                    # tail-only: borrow an (idle by then) spsum-tag slot so
                    # consecutive groups double-buffer instead of serializing
                    pso = sp.tile([P, 2 * ICH], F32, name="spsum",
                                  bufs=2)[:, 0:512]
                else:
                    pso = sp.tile([P, 512], F32, name="misc", bufs=1)
                for k in range(MT):
                    nc.tensor.matmul(
                        pso[:], at[k][:, mt * P:(mt + 1) * P],
                        wo_sb[:, k, nch2 * 512:(nch2 + 1) * 512],
                        start=(k == 0), stop=(k == MT - 1))
                ob = obp.tile([P, 512], F32, name="ob")
                nc.vector.tensor_copy(ob[:], pso[:])
                nc.sync.dma_start(
                    out_d[mt * P:(mt + 1) * P,
                          nch2 * 512:(nch2 + 1) * 512], ob[:])

            def emit_attn_chunk(ic, fillers=()):
                """Attention for query chunk ic, 4 head-pairs, with the PE
                stream software-pipelined (S of j-tile jt+1 ahead of AV of
                jt so exp latency is hidden) and fillers drained at an even
                per-slot rate to keep the PE busy while the scalar engine
                works through the exps. Fillers are (tag, fn); tags let the
                chunk force-drain a filler that a same-chunk instruction
                depends on (v3 tiles, same-chunk Q/K) — the tensor queue is
                in-order, so a dependency emitted after its consumer would
                deadlock."""
                isl = slice(ic * ICH, (ic + 1) * ICH)
                njt = 4 * ic + 4
                fillers = list(fillers)
                tag_idx = {tag: i for i, (tag, _) in enumerate(fillers)
                           if tag is not None}
                nslots = MT * njt
                fill_stride = max(1, -(-nslots // max(1, len(fillers))))
                slot = [0]
                drained = [0]
                pending = []

                def drain_to(i):
                    while drained[0] <= i:
                        fillers[drained[0]][1]()
                        drained[0] += 1

                def need(tag):
                    if tag in tag_idx:
                        drain_to(tag_idx[tag])

                def maybe_fill(n=1):
                    want = min(slot[0] // fill_stride + n - 1, len(fillers))
                    while drained[0] < want:
                        fillers[drained[0]][1]()
                        drained[0] += 1

                for hp in range(MT):
                    need(("QK", ic, hp, 0))
                    need(("QK", ic, hp, 1))
                    opsA = sp.tile([HD + 1, ICH], F32, name="opsum", bufs=3)
                    opsB = sp.tile([HD + 1, ICH], F32, name="opsum", bufs=3)
                    s2s, e2s = {}, {}

                    def emit_s(jt):
                        s2 = sp.tile([P, 2 * ICH], F32, name="spsum", bufs=2)
                        jsl = slice(jt * P, (jt + 1) * P)
                        c0 = max(jt - 4 * ic, 0) * P
                        qsl = slice(ic * ICH + c0, (ic + 1) * ICH)
                        nc.tensor.matmul(s2[:, c0:ICH],
                                         kt_[hp][0:HD, jsl],
                                         qt[hp][0:HD, qsl],
                                         start=True, stop=True)
                        nc.tensor.matmul(s2[:, ICH + c0:2 * ICH],
                                         kt_[hp][HD:P, jsl],
                                         qt[hp][HD:P, qsl],
                                         start=True, stop=True)
                        s2s[jt] = s2

                    def emit_exp(jt):
                        e2 = ap_.tile([P, 2 * ICH], BF16, name="e", bufs=3)
                        s2 = s2s.pop(jt)
                        kdiag = jt - 4 * ic
                        c0 = max(kdiag, 0) * P
                        if kdiag == 3:
                            # two small valid ranges; split beats one span
                            nc.scalar.activation(e2[:, c0:ICH],
                                                 s2[:, c0:ICH], AF.Exp)
                            nc.scalar.activation(e2[:, ICH + c0:2 * ICH],
                                                 s2[:, ICH + c0:2 * ICH],
                                                 AF.Exp)
                        else:
                            # single span from first valid col of head A to
                            # the end; covers head B's dead cols too but one
                            # ACT's fixed overhead beats two for small c0
                            nc.scalar.activation(e2[:, c0:2 * ICH],
                                                 s2[:, c0:2 * ICH], AF.Exp)
                        if kdiag >= 0:
                            # zero the diagonal block's upper triangle
                            for half in range(2):
                                o = half * ICH + c0
                                nc.vector.tensor_tensor(
                                    e2[:, o:o + P], e2[:, o:o + P],
                                    tri_sb[:], op=ALU.mult)
                        e2s[jt] = e2

                    def emit_av(jt):
                        # columns left of the diagonal block are causally
                        # invalid — restrict the accumulation to the valid
                        # column range instead of zeroing them
                        need(("V", jt))
                        kdiag = jt - 4 * ic
                        c0 = max(kdiag, 0) * P
                        e2 = e2s.pop(jt)
                        nc.tensor.matmul(opsA[:, c0:], v3[jt][:, 2 * hp, :],
                                         e2[:, c0:ICH],
                                         start=(jt == 0),
                                         stop=(jt == njt - 1))
                        nc.tensor.matmul(opsB[:, c0:],
                                         v3[jt][:, 2 * hp + 1, :],
                                         e2[:, ICH + c0:2 * ICH],
                                         start=(jt == 0),
                                         stop=(jt == njt - 1))

                    emit_s(0)
                    for jt in range(1, njt):
                        emit_s(jt)
                        if jt == 2 and pending:
                            pending.pop()()
                        emit_exp(jt - 1)
                        emit_av(jt - 1)
                        slot[0] += 1
                        maybe_fill()
                    emit_exp(njt - 1)
                    maybe_fill(2)
                    emit_av(njt - 1)
                    slot[0] += 1

                    if ic == 0 and pending:
                        pending.pop()()

                    def normalize(hp=hp, opsA=opsA, opsB=opsB):
                        # normalize straight out of PSUM: in0 is PSUM so the
                        # SBUF base-partition pairing rule doesn't apply
                        for half, ops in ((0, opsA), (1, opsB)):
                            po = half * HD
                            dn = ap_.tile([1, ICH], F32, name="dn", bufs=4)
                            nc.vector.tensor_copy(dn[:], ops[HD:HD + 1, :])
                            recip = ap_.tile([1, ICH], F32, name="recip",
                                             bufs=4)
                            nc.vector.reciprocal_approx_fast(recip[:], dn[:])
                            rb = ap_.tile([HD, ICH], F32, name="rb", bufs=4)
                            nc.gpsimd.partition_broadcast(rb[:], recip[:])
                            nc.vector.tensor_tensor(
                                at[hp][po:po + HD, isl], ops[0:HD, :], rb[:],
                                op=ALU.mult)

                    pending.append(normalize)

                while pending:
                    pending.pop()()
                for _, f in fillers[drained[0]:]:
                    f()

            def FQK(c, hp, which):
                return (("QK", c, hp, which),
                        lambda: emit_projqk(c, hp, which))

            def FV(tt):
                return (("V", tt), lambda: emit_v_tile(tt))

            def FO(mt, n):
                return (None, lambda: emit_out_group(mt, n))

            # chunk 0: remaining chunk-0 projections (deps: V(tt) before
            # AV of j-tile tt; Q/K(0,hp) before head-pair hp's S), then
            # ahead-of-time work for chunk 1
            f0 = [FV(0), FV(1), FQK(0, 1, 0), FQK(0, 1, 1), FV(2), FV(3),
                  FQK(0, 2, 0), FQK(0, 2, 1), FQK(0, 3, 0), FQK(0, 3, 1),
                  FQK(1, 0, 0), FQK(1, 0, 1), FQK(1, 1, 0), FQK(1, 1, 1),
                  FQK(1, 2, 0), FQK(1, 2, 1), FQK(1, 3, 0), FQK(1, 3, 1)]
            # chunk 1: V tiles 4-7 are needed by chunk 1 itself (hp0's
            # j-tiles 4-7) — schedule them first; then chunk-2 Q/K
            f1 = [FV(4), FV(5), FV(6), FV(7),
                  FQK(2, 0, 0), FQK(2, 0, 1), FQK(2, 1, 0), FQK(2, 1, 1),
                  FQK(2, 2, 0), FQK(2, 2, 1), FQK(2, 3, 0), FQK(2, 3, 1)]
            # chunk 2: V 8-11 needed here, chunk-3 Q/K, chunk-0 out-proj
            f2 = [FV(8), FV(9), FV(10), FV(11),
                  FQK(3, 0, 0), FQK(3, 0, 1), FQK(3, 1, 0), FQK(3, 1, 1),
                  FQK(3, 2, 0), FQK(3, 2, 1), FQK(3, 3, 0), FQK(3, 3, 1)]
            # chunk 3: V 12-15 needed here; out-proj of chunks 0-2 rides
            # in chunk 3's exp slack
            f3 = [FV(12), FV(13), FV(14), FV(15)] + \
                 [FO(mt, n) for mt in range(0, 12) for n in range(2)]

            emit_attn_chunk(0, f0)
            emit_attn_chunk(1, f1)
            emit_attn_chunk(2, f2)
            emit_attn_chunk(3, f3)
            for i, (mt, n) in enumerate(
                    (mt, n) for mt in range(12, 16) for n in range(2)):
                emit_out_group(mt, n, alt=(i % 2 == 1))

    nc.compile()
    return nc


def _get_program():
    global _PROGRAM
    if _PROGRAM is None:
        _install_ntff_hook()
        _PROGRAM = _build_program()
    return _PROGRAM


def _make_masks():
    """Multiplicative upper-triangle zero mask [128, 128] for the diagonal
    128x128 block of each S^T tile: entry (j, i) = 1 if j <= i else 0."""
    j = np.arange(P)[:, None]
    i = np.arange(P)[None, :]
    return (j <= i).astype(np.float32)


def make_in_maps(x, Wq, bq, Wk, bk, Wv, bv, Wo, bo):
    import ml_dtypes
    bf16 = ml_dtypes.bfloat16

    def sbl(a, k):
        """[k*128, n] -> SBUF layout [128, k*n] (partition-major runs)."""
        n = a.shape[1]
        return np.ascontiguousarray(
            a.reshape(k, P, n).transpose(1, 0, 2).reshape(P, k * n)
        ).astype(bf16)

    masks = _make_masks()
    in_maps = []
    for c in range(8):
        b, hg = c // 2, c % 2
        sl = slice(hg * QD, (hg + 1) * QD)
        in_maps.append({
            "xT": sbl(np.ascontiguousarray(x[b].T), KT),
            "wq": sbl(Wq[:, sl] * SCALE, KT),
            "wk": sbl(Wk[:, sl], KT),
            "wv": sbl(Wv[:, sl], KT),
            "wo": sbl(Wo[sl, :], MT),
            "bq": np.ascontiguousarray((bq[sl] * SCALE).reshape(MT, P).T),
            "bk": np.ascontiguousarray(bk[sl].reshape(MT, P).T),
            "bvb": np.ascontiguousarray(
                np.broadcast_to(bv[sl].astype(np.float32), (P, QD))),
            "msk": masks.astype(bf16),
        })
    return in_maps


def run(inputs, trace=False):
    from concourse.bass_utils import run_bass_kernel_spmd

    nc = _get_program()
    in_maps = make_in_maps(**inputs)
    res = run_bass_kernel_spmd(nc, in_maps, list(range(8)), trace=trace)
    bo = inputs["bo"]
    out = np.empty((B, T, D), dtype=np.float32)
    for b in range(B):
        out[b] = res.results[2 * b]["out"] + res.results[2 * b + 1]["out"] + bo
    return out, res


def kernel(**inputs):
    inputs = {k: np.asarray(v) for k, v in inputs.items()}
    out, _ = run(inputs)
    return out
